# revision 6
# baseline (speedup 1.0000x reference)
"""Trainium2 Bass kernel for nn_ChannelSpatialAttention2 (dense_cnn).

Data-parallel over batch: 16 samples / 8 cores = 2 samples per core, no
cross-core communication.  Per-sample dataflow (channel-major layout
(128 ch, 16384 px), px = h*128 + w):

  1. SP-queue HWDGE loads f_vi/f_ir fp32->bf16 in 4 quarters; pooled
     channel sums of (fv+fi) and (fv-fi) ride accum_out on DVE
     scalar_tensor_tensor scratch ops.
  2. Tiny MLP (1x1 convs + BN folded on host, +/- basis) -> channel
     weights a.
  3. conv1 folded:  g = relu((Wv + Wi*diag(a)) @ f_vi
                          + (Wi + Wv*diag(a)) @ f_ir + bc)
     with BN scale folded into the weights on host; avg-pool rides
     accum_out on the PSUM->SBUF evacuation (Act/DVE alternating).
  4. Per-quarter: DMA-xbar transpose g -> gt [w, h, c]; DVE 3D
     reduce_sum over c -> channel-mean map; DVE halving tree -> channel
     max map; partial pixel-max trees on g quarters.
  5. 7x7 spatial conv = 14 accumulating matmuls against host-built
     banded matrices -> sigmoid -> sa row.
  6. Blend per 1024-px chunk: tt = sigmoid(ca x sa) via K=1 rank-1
     matmuls + Act; dt = (fv-fi)*tt (DVE sub, DVE/Pool mul);
     out = diag(a)@fv + I@fi + diag(1-a)@dt accumulated in PSUM,
     evacuated bf16 (Act/DVE alternating), stored bf16 (host casts back
     to fp32).

  Sample 1's load/conv phases are software-pipelined into sample 0's
  blend phase (interleaved emission; per-engine program order is the
  schedule).
"""

import sys

if '/opt/trn_rl_repo' not in sys.path:
    sys.path.insert(0, '/opt/trn_rl_repo')

import numpy as np
import ml_dtypes

import concourse.bacc as bacc
import concourse.mybir as mybir
import concourse.tile as tile
import concourse.bass_utils as bass_utils

EPS = 1e-5
C = 128
N, H, W = 16, 128, 128
P = H * W            # 16384 pixels per sample
NCORES = 8
SPC = N // NCORES    # 2 samples per core
QP = P // 4          # 4096-px quarter
CH = 1024            # evac / blend chunk

BF16 = mybir.dt.bfloat16
F32 = mybir.dt.float32
AL = mybir.AluOpType
AF = mybir.ActivationFunctionType
AX = mybir.AxisListType

_cache = {}


def _build_program():
    nc = bacc.Bacc("TRN2", target_bir_lowering=False, debug=False,
                   enable_asserts=False, num_devices=NCORES)

    d_fvi = nc.dram_tensor("f_vi", (SPC, C, P), F32, kind="ExternalInput").ap()
    d_fir = nc.dram_tensor("f_ir", (SPC, C, P), F32, kind="ExternalInput").ap()
    d_out = nc.dram_tensor("out", (SPC, C, P), BF16, kind="ExternalOutput").ap()

    d_wvi = nc.dram_tensor("wvi_t", (C, C), BF16, kind="ExternalInput").ap()
    d_wir = nc.dram_tensor("wir_t", (C, C), BF16, kind="ExternalInput").ap()
    d_eye = nc.dram_tensor("eye", (C, C), BF16, kind="ExternalInput").ap()
    d_bmat = nc.dram_tensor("bmat", (14, 128, 128), BF16, kind="ExternalInput").ap()
    d_l1v = nc.dram_tensor("l1v", (C, C // 2), F32, kind="ExternalInput").ap()
    d_l1i = nc.dram_tensor("l1i", (C, C // 2), F32, kind="ExternalInput").ap()
    d_b1 = nc.dram_tensor("b1", (C // 2, 1), F32, kind="ExternalInput").ap()
    d_l2 = nc.dram_tensor("l2", (C // 2, C), F32, kind="ExternalInput").ap()
    d_b2 = nc.dram_tensor("b2", (C, 1), F32, kind="ExternalInput").ap()
    d_bc = nc.dram_tensor("bc", (C, 1), F32, kind="ExternalInput").ap()
    d_c1a = nc.dram_tensor("c1a", (C, 8), F32, kind="ExternalInput").ap()
    d_c1m = nc.dram_tensor("c1m", (C, 8), F32, kind="ExternalInput").ap()
    d_c2r = nc.dram_tensor("c2r", (8, C), F32, kind="ExternalInput").ap()
    d_one = nc.dram_tensor("one_col", (C, 1), BF16, kind="ExternalInput").ap()

    with tile.TileContext(nc) as tc:
        with (
            tc.tile_pool(name="wts", bufs=1) as wts,
            tc.tile_pool(name="io", bufs=6) as io,
            tc.tile_pool(name="gbuf", bufs=1) as gbuf,
            tc.tile_pool(name="gtb", bufs=2) as gtb,
            tc.tile_pool(name="scr", bufs=1) as scrp,
            tc.tile_pool(name="pxp", bufs=2) as pxp,
            tc.tile_pool(name="pads", bufs=2) as padp,
            tc.tile_pool(name="sm", bufs=2) as sm,
            tc.tile_pool(name="sfl", bufs=2) as sflp,
            tc.tile_pool(name="pxscr", bufs=2) as pxscr,
            tc.tile_pool(name="ttp", bufs=2) as ttp,
            tc.tile_pool(name="dtp", bufs=2) as dtp,
            tc.tile_pool(name="ob", bufs=3) as obp,
            tc.tile_pool(name="psA", bufs=2, space="PSUM") as psA,
            tc.tile_pool(name="psT", bufs=2, space="PSUM") as psT,
        ):
            # ---- constant loads (Pool SWDGE; overlaps the first input DMAs)
            l1v = wts.tile([C, C // 2], F32)
            l1i = wts.tile([C, C // 2], F32)
            b1 = wts.tile([C // 2, 1], F32)
            l2 = wts.tile([C // 2, C], F32)
            b2 = wts.tile([C, 1], F32)
            bc = wts.tile([C, 1], F32)
            wvi = wts.tile([C, C], BF16)
            wir = wts.tile([C, C], BF16)
            eye = wts.tile([C, C], BF16)
            c1a = wts.tile([C, 8], F32)
            c1m = wts.tile([C, 8], F32)
            c2r = wts.tile([8, C], F32)
            one_col = wts.tile([C, 1], BF16)
            bmat = wts.tile([128, 14, 128], BF16)
            for t, d in ((l1v, d_l1v), (l1i, d_l1i), (b1, d_b1), (l2, d_l2),
                         (b2, d_b2), (bc, d_bc), (wvi, d_wvi), (wir, d_wir),
                         (eye, d_eye), (c1a, d_c1a), (c1m, d_c1m),
                         (c2r, d_c2r), (one_col, d_one)):
                nc.sync.dma_start(t[:], d[:])
            nc.sync.dma_start(bmat[:], d_bmat.rearrange("m r c -> r m c"))

            # ---- per-sample state ----
            st = [dict() for _ in range(SPC)]

            def FV(s, sl):
                q = sl.start // QP
                o = sl.start % QP
                return st[s]['fv'][q][:, o:o + sl.stop - sl.start]

            def FI(s, sl):
                q = sl.start // QP
                o = sl.start % QP
                return st[s]['fi'][q][:, o:o + sl.stop - sl.start]

            def emit_alloc(s):
                d = st[s]
                d['fv'] = [None] * 4
                d['fi'] = [None] * 4
                d['spv'] = sm.tile([C, 8], F32, tag="spv", name=f"spv{s}")
                d['smv'] = sm.tile([C, 8], F32, tag="smv", name=f"smv{s}")
                d['avp'] = sm.tile([C, 16], F32, tag="avp", name=f"avp{s}")
                d['maxpad'] = padp.tile([128, 134], BF16, tag="maxpad",
                                        name=f"maxpad{s}")
                d['sumpad'] = padp.tile([128, 134], BF16, tag="sumpad",
                                        name=f"sumpad{s}")
                d['g'] = gbuf.tile([C, P], BF16, tag="g", name=f"g{s}")
                nc.vector.memset(d['maxpad'][:, 0:3], 0.0)
                nc.vector.memset(d['maxpad'][:, 131:134], 0.0)
                nc.vector.memset(d['sumpad'][:, 0:3], 0.0)
                nc.vector.memset(d['sumpad'][:, 131:134], 0.0)

            def emit_load_q(s, q):
                d = st[s]
                d['fv'][q] = io.tile([C, QP], BF16, tag="fv", name=f"fv{s}_{q}")
                d['fi'][q] = io.tile([C, QP], BF16, tag="fi", name=f"fi{s}_{q}")
                sl = slice(q * QP, (q + 1) * QP)
                nc.gpsimd.dma_start(d['fv'][q][:], d_fvi[s][:, sl])
                nc.gpsimd.dma_start(d['fi'][q][:], d_fir[s][:, sl])

            def emit_pool_q(s, q):
                # scratch sum/diff with pooled-channel accumulators riding
                d = st[s]
                for hh in range(2):
                    hs = slice(hh * 2048, (hh + 1) * 2048)
                    scr = scrp.tile([C, 2048], BF16, tag="scr",
                                    name=f"scr{s}_{q}_{hh}")
                    nc.vector.tensor_scalar(
                        scr[:], d['fv'][q][:, hs], 1.0, 0.0, AL.mult, AL.add,
                        accum_out=d['spv'][:, 2 * q + hh:2 * q + hh + 1])
                    nc.vector.tensor_scalar(
                        scr[:], d['fi'][q][:, hs], 1.0, 0.0, AL.mult, AL.add,
                        accum_out=d['smv'][:, 2 * q + hh:2 * q + hh + 1])

            def emit_amlp(s):
                d = st[s]
                sp = sm.tile([C, 1], F32, tag="sp", name=f"sp{s}")
                smn = sm.tile([C, 1], F32, tag="smn", name=f"smn{s}")
                nc.vector.reduce_sum(sp[:], d['spv'][:], axis=AX.X)
                nc.vector.reduce_sum(smn[:], d['smv'][:], axis=AX.X)
                ps1 = psA.tile([C, CH], F32, tag="big", name=f"ps1_{s}")
                nc.tensor.matmul(ps1[0:64, 0:1], l1v[:], sp[:], start=True, stop=False)
                nc.tensor.matmul(ps1[0:64, 0:1], l1i[:], smn[:], start=False, stop=True)
                h1 = sm.tile([C // 2, 1], F32, tag="h1", name=f"h1_{s}")
                nc.scalar.activation(h1[:], ps1[0:64, 0:1], AF.Relu, bias=b1[:])
                ps2 = psA.tile([C, CH], F32, tag="big", name=f"ps2_{s}")
                nc.tensor.matmul(ps2[0:C, 0:1], l2[:], h1[:], start=True, stop=True)
                a_col = sm.tile([C, 1], F32, tag="a_col", name=f"a_col{s}")
                nc.scalar.activation(a_col[:], ps2[0:C, 0:1], AF.Sigmoid, bias=b2[:])
                oma = sm.tile([C, 1], F32, tag="oma", name=f"oma{s}")
                nc.vector.tensor_scalar(oma[:], a_col[:], -1.0, 1.0, AL.mult, AL.add)
                lv = sm.tile([C, C], BF16, tag="lv", name=f"lv{s}")
                li = sm.tile([C, C], BF16, tag="li", name=f"li{s}")
                dga = sm.tile([C, C], BF16, tag="dga", name=f"dga{s}")
                dgo = sm.tile([C, C], BF16, tag="dgo", name=f"dgo{s}")
                nc.vector.scalar_tensor_tensor(lv[:], wir[:], a_col[:], wvi[:], AL.mult, AL.add)
                nc.vector.scalar_tensor_tensor(li[:], wvi[:], a_col[:], wir[:], AL.mult, AL.add)
                nc.vector.tensor_scalar(dga[:], eye[:], a_col[:], 0.0, AL.mult, AL.add)
                nc.vector.tensor_scalar(dgo[:], eye[:], oma[:], 0.0, AL.mult, AL.add)
                d['lv'], d['li'], d['dga'], d['dgo'] = lv, li, dga, dgo

            def emit_conv_chunk(s, c):
                # g chunk c (1024 px) = relu(lv@fv + li@fi + bc), avg rides
                d = st[s]
                pg = psA.tile([C, CH], F32, tag="big", name=f"pg{s}_{c}")
                for hf in range(2):
                    sl = slice(c * CH + hf * 512, c * CH + (hf + 1) * 512)
                    po = pg[:, hf * 512:(hf + 1) * 512]
                    nc.tensor.matmul(po, d['lv'][:], FV(s, sl), start=True, stop=False)
                for hf in range(2):
                    sl = slice(c * CH + hf * 512, c * CH + (hf + 1) * 512)
                    po = pg[:, hf * 512:(hf + 1) * 512]
                    nc.tensor.matmul(po, d['li'][:], FI(s, sl), start=False, stop=True)
                gch = d['g'][:, c * CH:(c + 1) * CH]
                acc = d['avp'][:, c:c + 1]
                nc.scalar.activation(gch, pg[:], AF.Relu, bias=bc[:],
                                     accum_out=acc)

            def emit_mean_q(s, q):
                # channel-sum columns via 1-col matmuls (sum over partitions)
                d = st[s]
                if q == 0:
                    d['psmm'] = psT.tile([C, CH], F32, tag="tt", name=f"psmm{s}")
                for hh in range(q * 32, q * 32 + 32):
                    nc.tensor.matmul(
                        d['psmm'][:, hh:hh + 1],
                        d['g'][:, hh * 128:(hh + 1) * 128], one_col[:],
                        start=True, stop=True)

            def emit_mean_evac(s):
                d = st[s]
                nc.scalar.activation(d['sumpad'][:, 3:131], d['psmm'][:, 0:128],
                                     AF.Copy)

            def emit_stats_q(s, q):
                d = st[s]
                gq = d['g'][:, q * QP:(q + 1) * QP]
                # pixel-max partials (non-destructive)
                eng = nc.vector
                if q == 0:
                    px = pxp.tile([C, QP // 2], BF16, tag="px", name=f"px{s}")
                    d['px'] = px
                    eng.tensor_tensor(out=px[:], in0=gq[:, 0:QP // 2],
                                      in1=gq[:, QP // 2:QP], op=AL.max)
                else:
                    tsc = pxscr.tile([C, 2048], BF16, tag="pxscr",
                                     name=f"tsc{s}_{q}")
                    nc.vector.tensor_tensor(out=tsc[:],
                                            in0=gq[:, 0:QP // 2],
                                            in1=gq[:, QP // 2:QP], op=AL.max)
                    eng.tensor_tensor(out=d['px'][:], in0=d['px'][:],
                                      in1=tsc[:], op=AL.max)
                # transpose quarter -> [w, h_local, c]
                gt = gtb.tile([128, QP], BF16, tag="gt", name=f"gt{s}_{q}")
                gt3 = gt[:].rearrange("p (h c) -> p h c", c=128)
                nc.sync.dma_start_transpose(gt3, gq)
                csc = scrp.tile([C, 2048], BF16, tag="scr", name=f"csc{s}_{q}")
                csc3 = csc[:].rearrange("p (h c) -> p h c", c=64)
                nc.vector.tensor_tensor(
                    out=csc3[:], in0=gt3[:, :, 0:64],
                    in1=gt3[:, :, 64:128], op=AL.max)
                w_ = 32
                while w_ >= 1:
                    nc.vector.tensor_tensor(
                        out=csc3[:, :, 0:w_], in0=csc3[:, :, 0:w_],
                        in1=csc3[:, :, w_:2 * w_], op=AL.max)
                    w_ //= 2
                nc.vector.tensor_copy(
                    d['maxpad'][:, 3 + q * 32:35 + q * 32].rearrange(
                        "p (f o) -> p f o", o=1),
                    csc3[:, :, 0:1])

            def emit_stats_tail(s):
                d = st[s]
                # finish pixel max: 2048 -> 64 -> 1
                w_ = 1024
                while w_ >= 64:
                    nc.vector.tensor_tensor(out=d['px'][:, 0:w_],
                                            in0=d['px'][:, 0:w_],
                                            in1=d['px'][:, w_:2 * w_], op=AL.max)
                    w_ //= 2
                mx = sm.tile([C, 1], F32, tag="mx", name=f"mx{s}")
                nc.vector.reduce_max(mx[:], d['px'][:, 0:64], axis=AX.X)
                avg = sm.tile([C, 1], F32, tag="avg", name=f"avg{s}")
                nc.vector.reduce_sum(avg[:], d['avp'][:], axis=AX.X)
                # ChannelAttention MLP
                psa = psA.tile([C, CH], F32, tag="big", name=f"psa{s}")
                nc.tensor.matmul(psa[0:8, 0:1], c1a[:], avg[:], start=True, stop=True)
                ha = sm.tile([8, 1], F32, tag="ha", name=f"ha{s}")
                nc.scalar.activation(ha[:], psa[0:8, 0:1], AF.Relu)
                psm = psA.tile([C, CH], F32, tag="big", name=f"psm{s}")
                nc.tensor.matmul(psm[0:8, 0:1], c1m[:], mx[:], start=True, stop=True)
                hm = sm.tile([8, 1], F32, tag="hm", name=f"hm{s}")
                nc.scalar.activation(hm[:], psm[0:8, 0:1], AF.Relu)
                psr = psA.tile([C, CH], F32, tag="big", name=f"psr{s}")
                nc.tensor.matmul(psr[0:1, 0:C], ha[:], c2r[:], start=True, stop=False)
                nc.tensor.matmul(psr[0:1, 0:C], hm[:], c2r[:], start=False, stop=True)
                ca = sm.tile([1, C], BF16, tag="ca", name=f"ca{s}")
                nc.scalar.activation(ca[:], psr[0:1, 0:C], AF.Sigmoid)
                d['ca'] = ca
                # SpatialAttention 7x7 as banded matmuls
                pss = psA.tile([C, CH], F32, tag="big", name=f"pss{s}")
                first = True
                for chn, pad in ((0, d['sumpad']), (1, d['maxpad'])):
                    for dy in range(7):
                        nc.tensor.matmul(pss[0:128, 0:128], pad[:, dy:dy + 128],
                                         bmat[:, chn * 7 + dy, :],
                                         start=first,
                                         stop=(chn == 1 and dy == 6))
                        first = False
                sa_hw = sm.tile([128, 128], BF16, tag="sa_hw", name=f"sa_hw{s}")
                nc.scalar.activation(sa_hw[:], pss[0:128, 0:128], AF.Sigmoid)
                d['sa_hw'] = sa_hw
                d['sa_fl'] = [None] * 4

            def emit_sa_flatten(s, e):
                d = st[s]
                sfq = sflp.tile([1, QP], BF16, tag="sa_fl",
                                name=f"sa_fl{s}_{e}")
                nc.sync.dma_start(sfq[:], d['sa_hw'][32 * e:32 * (e + 1), :])
                d['sa_fl'][e] = sfq

            def emit_blend_chunk(s, k):
                d = st[s]
                slf = slice(k * CH, (k + 1) * CH)
                sfq = d['sa_fl'][k // 4]
                so = (k % 4) * CH
                # tt = sigmoid(ca (x) sa) rank-1
                ptt = psT.tile([C, CH], F32, tag="tt", name=f"ptt{s}_{k}")
                nc.tensor.matmul(ptt[:, 0:512], d['ca'][:],
                                 sfq[:, so:so + 512],
                                 start=True, stop=True)
                nc.tensor.matmul(ptt[:, 512:1024], d['ca'][:],
                                 sfq[:, so + 512:so + 1024],
                                 start=True, stop=True)
                tt = ttp.tile([C, CH], BF16, tag="tt_sb", name=f"tt{s}_{k}")
                nc.scalar.activation(tt[:], ptt[:], AF.Sigmoid)
                # dt = (fv - fi) * tt
                dtmp = dtp.tile([C, CH], BF16, tag="dtmp", name=f"dtmp{s}_{k}")
                nc.vector.tensor_tensor(out=dtmp[:], in0=FV(s, slf),
                                        in1=FI(s, slf), op=AL.subtract)
                pool_mul = False
                if pool_mul:
                    nc.gpsimd.tensor_tensor(out=dtmp[:], in0=dtmp[:], in1=tt[:],
                                            op=AL.mult)
                else:
                    nc.vector.tensor_tensor(out=dtmp[:], in0=dtmp[:], in1=tt[:],
                                            op=AL.mult)
                # out = dga@fv + I@fi + dgo@dt
                pb = psA.tile([C, CH], F32, tag="big", name=f"pb{s}_{k}")
                for hf in range(2):
                    fs = slice(k * CH + hf * 512, k * CH + (hf + 1) * 512)
                    po = pb[:, hf * 512:(hf + 1) * 512]
                    nc.tensor.matmul(po, d['dga'][:], FV(s, fs), start=True, stop=False)
                for hf in range(2):
                    fs = slice(k * CH + hf * 512, k * CH + (hf + 1) * 512)
                    po = pb[:, hf * 512:(hf + 1) * 512]
                    nc.tensor.matmul(po, eye[:], FI(s, fs), start=False, stop=False)
                for hf in range(2):
                    ms = slice(hf * 512, (hf + 1) * 512)
                    po = pb[:, hf * 512:(hf + 1) * 512]
                    nc.tensor.matmul(po, d['dgo'][:], dtmp[:, ms], start=False, stop=True)
                ob = obp.tile([C, CH], BF16, tag="ob", name=f"ob{s}_{k}")
                act_evac = not (s == 1 and k % 4 == 2)
                if act_evac:
                    nc.scalar.copy(ob[:], pb[:])
                else:
                    nc.vector.tensor_scalar(ob[:], pb[:], 1.0, 0.0, AL.mult, AL.add)
                nc.sync.dma_start(d_out[s][:, slf], ob[:])

            # ================= schedule =================
            emit_alloc(0)
            for q in range(4):
                emit_load_q(0, q)
                emit_pool_q(0, q)
            emit_amlp(0)
            emit_alloc(1)
            emit_load_q(1, 0)
            emit_load_q(1, 1)
            for q in range(4):
                for c in range(4 * q, 4 * q + 4):
                    emit_conv_chunk(0, c)
                emit_mean_q(0, q)
                emit_stats_q(0, q)
            emit_pool_q(1, 0)
            emit_pool_q(1, 1)
            emit_load_q(1, 2)
            emit_pool_q(1, 2)
            emit_load_q(1, 3)
            emit_pool_q(1, 3)
            emit_mean_evac(0)
            emit_stats_tail(0)
            emit_sa_flatten(0, 0)
            emit_sa_flatten(0, 1)
            for k in range(0, 4):
                emit_blend_chunk(0, k)
            emit_sa_flatten(0, 2)
            for k in range(4, 8):
                emit_blend_chunk(0, k)
            emit_sa_flatten(0, 3)
            emit_blend_chunk(0, 8)
            emit_amlp(1)
            emit_blend_chunk(0, 9)
            emit_blend_chunk(0, 10)
            cq = 0
            for k in range(11, 16):
                if cq < 4:
                    for c in range(4 * cq, 4 * cq + 4):
                        emit_conv_chunk(1, c)
                    emit_stats_q(1, cq)
                    cq += 1
                emit_blend_chunk(0, k)
            while cq < 4:
                for c in range(4 * cq, 4 * cq + 4):
                    emit_conv_chunk(1, c)
                emit_stats_q(1, cq)
                cq += 1
            for q in range(4):
                emit_mean_q(1, q)
            emit_mean_evac(1)
            emit_stats_tail(1)
            emit_sa_flatten(1, 0)
            emit_sa_flatten(1, 1)
            for k in range(16):
                emit_blend_chunk(1, k)
                if k == 3:
                    emit_sa_flatten(1, 2)
                if k == 7:
                    emit_sa_flatten(1, 3)

    nc.compile()
    return nc


def _host_consts(ca1_w, ca1_b, bn_a_g, bn_a_b, bn_a_m, bn_a_v,
                 ca2_w, ca2_b, bn_b_g, bn_b_b, bn_b_m, bn_b_v,
                 conv1_w, conv1_b, bn_c_g, bn_c_b, bn_c_m, bn_c_v,
                 chatt_w1, chatt_w2, sa_w):
    bf = ml_dtypes.bfloat16
    f = np.float32
    k_a = bn_a_g / np.sqrt(bn_a_v + EPS)
    w1 = ca1_w * k_a[:, None]
    b1 = (ca1_b - bn_a_m) * k_a + bn_a_b
    k_b = bn_b_g / np.sqrt(bn_b_v + EPS)
    w2 = ca2_w * k_b[:, None]
    b2 = (ca2_b - bn_b_m) * k_b + bn_b_b
    s_c = bn_c_g / np.sqrt(bn_c_v + EPS)
    b_c = (conv1_b - bn_c_m) * s_c + bn_c_b
    # conv1 weights with BN scale folded (scales output channel o)
    wv = conv1_w[:, :C] * s_c[:, None]
    wi = conv1_w[:, C:] * s_c[:, None]
    l1v = (w1[:, :C] / P).T
    l1i = (w1[:, C:] / P).T
    bmat = np.zeros((14, 128, 128), np.float32)
    for chn in range(2):
        scale = (1.0 / 128.0) if chn == 0 else 1.0
        for dy in range(7):
            for dx in range(7):
                off = dx - 3
                v = sa_w[0, chn, dy, dx] * scale
                if off >= 0:
                    idx = np.arange(0, 128 - off)
                    bmat[chn * 7 + dy, idx + off, idx] = v
                else:
                    idx = np.arange(-off, 128)
                    bmat[chn * 7 + dy, idx + off, idx] = v
    return {
        "wvi_t": np.ascontiguousarray(wv.T).astype(bf),
        "wir_t": np.ascontiguousarray(wi.T).astype(bf),
        "eye": np.eye(C, dtype=f).astype(bf),
        "bmat": bmat.astype(bf),
        "l1v": np.ascontiguousarray(l1v).astype(f),
        "l1i": np.ascontiguousarray(l1i).astype(f),
        "b1": b1.reshape(-1, 1).astype(f),
        "l2": np.ascontiguousarray(w2.T).astype(f),
        "b2": b2.reshape(-1, 1).astype(f),
        "bc": b_c.reshape(-1, 1).astype(f),
        "c1a": np.ascontiguousarray((chatt_w1 / P).T).astype(f),
        "c1m": np.ascontiguousarray(chatt_w1.T).astype(f),
        "c2r": np.ascontiguousarray(chatt_w2.T).astype(f),
        "one_col": np.ones((C, 1), f).astype(bf),
    }


def kernel(f_vi, f_ir, ca1_w, ca1_b, bn_a_g, bn_a_b, bn_a_m, bn_a_v,
           ca2_w, ca2_b, bn_b_g, bn_b_b, bn_b_m, bn_b_v,
           conv1_w, conv1_b, bn_c_g, bn_c_b, bn_c_m, bn_c_v,
           chatt_w1, chatt_w2, sa_w, _trace=False):
    if "nc" not in _cache:
        _cache["nc"] = _build_program()
    nc = _cache["nc"]

    consts = _host_consts(
        np.asarray(ca1_w, np.float32), np.asarray(ca1_b, np.float32),
        np.asarray(bn_a_g, np.float32), np.asarray(bn_a_b, np.float32),
        np.asarray(bn_a_m, np.float32), np.asarray(bn_a_v, np.float32),
        np.asarray(ca2_w, np.float32), np.asarray(ca2_b, np.float32),
        np.asarray(bn_b_g, np.float32), np.asarray(bn_b_b, np.float32),
        np.asarray(bn_b_m, np.float32), np.asarray(bn_b_v, np.float32),
        np.asarray(conv1_w, np.float32), np.asarray(conv1_b, np.float32),
        np.asarray(bn_c_g, np.float32), np.asarray(bn_c_b, np.float32),
        np.asarray(bn_c_m, np.float32), np.asarray(bn_c_v, np.float32),
        np.asarray(chatt_w1, np.float32), np.asarray(chatt_w2, np.float32),
        np.asarray(sa_w, np.float32))

    fv = np.asarray(f_vi, np.float32).reshape(N, C, P)
    fi = np.asarray(f_ir, np.float32).reshape(N, C, P)
    in_maps = []
    for i in range(NCORES):
        m = dict(consts)
        m["f_vi"] = np.ascontiguousarray(fv[i * SPC:(i + 1) * SPC])
        m["f_ir"] = np.ascontiguousarray(fi[i * SPC:(i + 1) * SPC])
        in_maps.append(m)

    res = bass_utils.run_bass_kernel_spmd(nc, in_maps, core_ids=list(range(NCORES)),
                                          trace=_trace)
    if _trace:
        _cache["last_trace"] = res
    out = np.concatenate(
        [np.asarray(res.results[i]["out"]).astype(np.float32)
         for i in range(NCORES)], axis=0)
    return out.reshape(N, C, H, W)



# revision 7
# speedup vs baseline: 1.0375x; 1.0375x over previous
"""Trainium2 Bass kernel for nn_ChannelSpatialAttention2 (dense_cnn).

Data-parallel over batch: 16 samples / 8 cores = 2 samples per core, no
cross-core communication.  Per-sample dataflow (channel-major layout
(128 ch, 16384 px), px = h*128 + w):

  1. SP-queue HWDGE loads f_vi/f_ir fp32->bf16 in 4 quarters; pooled
     channel sums of (fv+fi) and (fv-fi) ride accum_out on DVE
     scalar_tensor_tensor scratch ops.
  2. Tiny MLP (1x1 convs + BN folded on host, +/- basis) -> channel
     weights a.
  3. conv1 folded:  g = relu((Wv + Wi*diag(a)) @ f_vi
                          + (Wi + Wv*diag(a)) @ f_ir + bc)
     with BN scale folded into the weights on host; avg-pool rides
     accum_out on the PSUM->SBUF evacuation (Act/DVE alternating).
  4. Per-quarter: DMA-xbar transpose g -> gt [w, h, c]; DVE 3D
     reduce_sum over c -> channel-mean map; DVE halving tree -> channel
     max map; partial pixel-max trees on g quarters.
  5. 7x7 spatial conv = 14 accumulating matmuls against host-built
     banded matrices -> sigmoid -> sa row.
  6. Blend per 1024-px chunk: tt = sigmoid(ca x sa) via K=1 rank-1
     matmuls + Act; dt = (fv-fi)*tt (DVE sub, DVE/Pool mul);
     out = diag(a)@fv + I@fi + diag(1-a)@dt accumulated in PSUM,
     evacuated bf16 (Act/DVE alternating), stored bf16 (host casts back
     to fp32).

  Sample 1's load/conv phases are software-pipelined into sample 0's
  blend phase (interleaved emission; per-engine program order is the
  schedule).
"""

import sys

if '/opt/trn_rl_repo' not in sys.path:
    sys.path.insert(0, '/opt/trn_rl_repo')

import numpy as np
import ml_dtypes

import concourse.bacc as bacc
import concourse.mybir as mybir
import concourse.tile as tile
import concourse.bass_utils as bass_utils

EPS = 1e-5
C = 128
N, H, W = 16, 128, 128
P = H * W            # 16384 pixels per sample
NCORES = 8
SPC = N // NCORES    # 2 samples per core
QP = P // 4          # 4096-px quarter
CH = 1024            # evac / blend chunk

BF16 = mybir.dt.bfloat16
F32 = mybir.dt.float32
AL = mybir.AluOpType
AF = mybir.ActivationFunctionType
AX = mybir.AxisListType

_cache = {}


def _build_program():
    nc = bacc.Bacc("TRN2", target_bir_lowering=False, debug=False,
                   enable_asserts=False, num_devices=NCORES)

    d_fvi = nc.dram_tensor("f_vi", (SPC, C, P), F32, kind="ExternalInput").ap()
    d_fir = nc.dram_tensor("f_ir", (SPC, C, P), F32, kind="ExternalInput").ap()
    d_out = nc.dram_tensor("out", (SPC, C, P), BF16, kind="ExternalOutput").ap()

    d_wvi = nc.dram_tensor("wvi_t", (C, C), BF16, kind="ExternalInput").ap()
    d_wir = nc.dram_tensor("wir_t", (C, C), BF16, kind="ExternalInput").ap()
    d_eye = nc.dram_tensor("eye", (C, C), BF16, kind="ExternalInput").ap()
    d_bmat = nc.dram_tensor("bmat", (14, 128, 128), BF16, kind="ExternalInput").ap()
    d_l1v = nc.dram_tensor("l1v", (C, C // 2), F32, kind="ExternalInput").ap()
    d_l1i = nc.dram_tensor("l1i", (C, C // 2), F32, kind="ExternalInput").ap()
    d_b1 = nc.dram_tensor("b1", (C // 2, 1), F32, kind="ExternalInput").ap()
    d_l2 = nc.dram_tensor("l2", (C // 2, C), F32, kind="ExternalInput").ap()
    d_b2 = nc.dram_tensor("b2", (C, 1), F32, kind="ExternalInput").ap()
    d_bc = nc.dram_tensor("bc", (C, 1), F32, kind="ExternalInput").ap()
    d_c1a = nc.dram_tensor("c1a", (C, 8), F32, kind="ExternalInput").ap()
    d_c1m = nc.dram_tensor("c1m", (C, 8), F32, kind="ExternalInput").ap()
    d_c2r = nc.dram_tensor("c2r", (8, C), F32, kind="ExternalInput").ap()
    d_one = nc.dram_tensor("one_col", (C, 1), BF16, kind="ExternalInput").ap()

    with tile.TileContext(nc) as tc:
        with (
            tc.tile_pool(name="wts", bufs=1) as wts,
            tc.tile_pool(name="io", bufs=6) as io,
            tc.tile_pool(name="gbuf", bufs=1) as gbuf,
            tc.tile_pool(name="gtb", bufs=2) as gtb,
            tc.tile_pool(name="scr", bufs=1) as scrp,
            tc.tile_pool(name="pxp", bufs=2) as pxp,
            tc.tile_pool(name="pads", bufs=2) as padp,
            tc.tile_pool(name="sm", bufs=2) as sm,
            tc.tile_pool(name="sfl", bufs=2) as sflp,
            tc.tile_pool(name="pxscr", bufs=2) as pxscr,
            tc.tile_pool(name="ttp", bufs=2) as ttp,
            tc.tile_pool(name="dtp", bufs=2) as dtp,
            tc.tile_pool(name="ob", bufs=3) as obp,
            tc.tile_pool(name="psA", bufs=2, space="PSUM") as psA,
            tc.tile_pool(name="psT", bufs=2, space="PSUM") as psT,
        ):
            # ---- constant loads (Pool SWDGE; overlaps the first input DMAs)
            l1v = wts.tile([C, C // 2], F32)
            l1i = wts.tile([C, C // 2], F32)
            b1 = wts.tile([C // 2, 1], F32)
            l2 = wts.tile([C // 2, C], F32)
            b2 = wts.tile([C, 1], F32)
            bc = wts.tile([C, 1], F32)
            wvi = wts.tile([C, C], BF16)
            wir = wts.tile([C, C], BF16)
            eye = wts.tile([C, C], BF16)
            c1a = wts.tile([C, 8], F32)
            c1m = wts.tile([C, 8], F32)
            c2r = wts.tile([8, C], F32)
            one_col = wts.tile([C, 1], BF16)
            bmat = wts.tile([128, 14, 128], BF16)
            for t, d in ((l1v, d_l1v), (l1i, d_l1i), (b1, d_b1), (l2, d_l2),
                         (b2, d_b2), (bc, d_bc), (wvi, d_wvi), (wir, d_wir),
                         (eye, d_eye), (c1a, d_c1a), (c1m, d_c1m),
                         (c2r, d_c2r), (one_col, d_one)):
                nc.sync.dma_start(t[:], d[:])
            nc.sync.dma_start(bmat[:], d_bmat.rearrange("m r c -> r m c"))

            # ---- per-sample state ----
            st = [dict() for _ in range(SPC)]

            def FV(s, sl):
                q = sl.start // QP
                o = sl.start % QP
                return st[s]['fv'][q][:, o:o + sl.stop - sl.start]

            def FI(s, sl):
                q = sl.start // QP
                o = sl.start % QP
                return st[s]['fi'][q][:, o:o + sl.stop - sl.start]

            def emit_alloc(s):
                d = st[s]
                d['fv'] = [None] * 4
                d['fi'] = [None] * 4
                d['spv'] = sm.tile([C, 8], F32, tag="spv", name=f"spv{s}")
                d['smv'] = sm.tile([C, 8], F32, tag="smv", name=f"smv{s}")
                d['avp'] = sm.tile([C, 16], F32, tag="avp", name=f"avp{s}")
                d['maxpad'] = padp.tile([128, 134], BF16, tag="maxpad",
                                        name=f"maxpad{s}")
                d['sumpad'] = padp.tile([128, 134], BF16, tag="sumpad",
                                        name=f"sumpad{s}")
                d['g'] = gbuf.tile([C, P], BF16, tag="g", name=f"g{s}")
                nc.vector.memset(d['maxpad'][:, 0:3], 0.0)
                nc.vector.memset(d['maxpad'][:, 131:134], 0.0)
                nc.vector.memset(d['sumpad'][:, 0:3], 0.0)
                nc.vector.memset(d['sumpad'][:, 131:134], 0.0)

            def emit_load_q(s, q):
                d = st[s]
                d['fv'][q] = io.tile([C, QP], BF16, tag="fv", name=f"fv{s}_{q}")
                d['fi'][q] = io.tile([C, QP], BF16, tag="fi", name=f"fi{s}_{q}")
                sl = slice(q * QP, (q + 1) * QP)
                nc.gpsimd.dma_start(d['fv'][q][:], d_fvi[s][:, sl])
                nc.gpsimd.dma_start(d['fi'][q][:], d_fir[s][:, sl])

            def emit_pool_q(s, q):
                # scratch sum/diff with pooled-channel accumulators riding
                d = st[s]
                for hh in range(2):
                    hs = slice(hh * 2048, (hh + 1) * 2048)
                    scr = scrp.tile([C, 2048], BF16, tag="scr",
                                    name=f"scr{s}_{q}_{hh}")
                    nc.vector.tensor_scalar(
                        scr[:], d['fv'][q][:, hs], 1.0, 0.0, AL.mult, AL.add,
                        accum_out=d['spv'][:, 2 * q + hh:2 * q + hh + 1])
                    nc.vector.tensor_scalar(
                        scr[:], d['fi'][q][:, hs], 1.0, 0.0, AL.mult, AL.add,
                        accum_out=d['smv'][:, 2 * q + hh:2 * q + hh + 1])

            def emit_amlp(s):
                d = st[s]
                sp = sm.tile([C, 1], F32, tag="sp", name=f"sp{s}")
                smn = sm.tile([C, 1], F32, tag="smn", name=f"smn{s}")
                nc.vector.reduce_sum(sp[:], d['spv'][:], axis=AX.X)
                nc.vector.reduce_sum(smn[:], d['smv'][:], axis=AX.X)
                ps1 = psA.tile([C, CH], F32, tag="big", name=f"ps1_{s}")
                nc.tensor.matmul(ps1[0:64, 0:1], l1v[:], sp[:], start=True, stop=False)
                nc.tensor.matmul(ps1[0:64, 0:1], l1i[:], smn[:], start=False, stop=True)
                h1 = sm.tile([C // 2, 1], F32, tag="h1", name=f"h1_{s}")
                nc.scalar.activation(h1[:], ps1[0:64, 0:1], AF.Relu, bias=b1[:])
                ps2 = psA.tile([C, CH], F32, tag="big", name=f"ps2_{s}")
                nc.tensor.matmul(ps2[0:C, 0:1], l2[:], h1[:], start=True, stop=True)
                a_col = sm.tile([C, 1], F32, tag="a_col", name=f"a_col{s}")
                nc.scalar.activation(a_col[:], ps2[0:C, 0:1], AF.Sigmoid, bias=b2[:])
                oma = sm.tile([C, 1], F32, tag="oma", name=f"oma{s}")
                nc.vector.tensor_scalar(oma[:], a_col[:], -1.0, 1.0, AL.mult, AL.add)
                lv = sm.tile([C, C], BF16, tag="lv", name=f"lv{s}")
                li = sm.tile([C, C], BF16, tag="li", name=f"li{s}")
                dga = sm.tile([C, C], BF16, tag="dga", name=f"dga{s}")
                dgo = sm.tile([C, C], BF16, tag="dgo", name=f"dgo{s}")
                nc.vector.scalar_tensor_tensor(lv[:], wir[:], a_col[:], wvi[:], AL.mult, AL.add)
                nc.vector.scalar_tensor_tensor(li[:], wvi[:], a_col[:], wir[:], AL.mult, AL.add)
                nc.vector.tensor_scalar(dga[:], eye[:], a_col[:], 0.0, AL.mult, AL.add)
                nc.vector.tensor_scalar(dgo[:], eye[:], oma[:], 0.0, AL.mult, AL.add)
                d['lv'], d['li'], d['dga'], d['dgo'] = lv, li, dga, dgo

            def emit_conv_chunk(s, c):
                # g chunk c (1024 px) = relu(lv@fv + li@fi + bc), avg rides
                d = st[s]
                pg = psA.tile([C, CH], F32, tag="big", name=f"pg{s}_{c}")
                for hf in range(2):
                    sl = slice(c * CH + hf * 512, c * CH + (hf + 1) * 512)
                    po = pg[:, hf * 512:(hf + 1) * 512]
                    nc.tensor.matmul(po, d['lv'][:], FV(s, sl), start=True, stop=False)
                for hf in range(2):
                    sl = slice(c * CH + hf * 512, c * CH + (hf + 1) * 512)
                    po = pg[:, hf * 512:(hf + 1) * 512]
                    nc.tensor.matmul(po, d['li'][:], FI(s, sl), start=False, stop=True)
                gch = d['g'][:, c * CH:(c + 1) * CH]
                acc = d['avp'][:, c:c + 1]
                if s == 0 or c % 2 == 0:
                    nc.scalar.activation(gch, pg[:], AF.Relu, bias=bc[:],
                                         accum_out=acc)
                else:
                    nc.vector.tensor_scalar(gch, pg[:], bc[:], 0.0,
                                            AL.add, AL.max, accum_out=acc)

            def emit_mean_q(s, q):
                # channel-sum columns via 1-col matmuls (sum over partitions)
                d = st[s]
                if q == 0:
                    d['psmm'] = psT.tile([C, CH], F32, tag="tt", name=f"psmm{s}")
                for hh in range(q * 32, q * 32 + 32):
                    nc.tensor.matmul(
                        d['psmm'][:, hh:hh + 1],
                        d['g'][:, hh * 128:(hh + 1) * 128], one_col[:],
                        start=True, stop=True)

            def emit_mean_evac(s):
                d = st[s]
                nc.scalar.activation(d['sumpad'][:, 3:131], d['psmm'][:, 0:128],
                                     AF.Copy)

            def emit_stats_q(s, q):
                d = st[s]
                gq = d['g'][:, q * QP:(q + 1) * QP]
                # pixel-max partials (non-destructive)
                eng = nc.vector
                if q == 0:
                    px = pxp.tile([C, QP // 2], BF16, tag="px", name=f"px{s}")
                    d['px'] = px
                    eng.tensor_tensor(out=px[:], in0=gq[:, 0:QP // 2],
                                      in1=gq[:, QP // 2:QP], op=AL.max)
                else:
                    tsc = pxscr.tile([C, 2048], BF16, tag="pxscr",
                                     name=f"tsc{s}_{q}")
                    nc.vector.tensor_tensor(out=tsc[:],
                                            in0=gq[:, 0:QP // 2],
                                            in1=gq[:, QP // 2:QP], op=AL.max)
                    eng.tensor_tensor(out=d['px'][:], in0=d['px'][:],
                                      in1=tsc[:], op=AL.max)
                # transpose quarter -> [w, h_local, c]
                gt = gtb.tile([128, QP], BF16, tag="gt", name=f"gt{s}_{q}")
                gt3 = gt[:].rearrange("p (h c) -> p h c", c=128)
                nc.sync.dma_start_transpose(gt3, gq)
                csc = scrp.tile([C, 2048], BF16, tag="scr", name=f"csc{s}_{q}")
                csc3 = csc[:].rearrange("p (h c) -> p h c", c=64)
                nc.vector.tensor_tensor(
                    out=csc3[:], in0=gt3[:, :, 0:64],
                    in1=gt3[:, :, 64:128], op=AL.max)
                w_ = 32
                while w_ >= 1:
                    nc.vector.tensor_tensor(
                        out=csc3[:, :, 0:w_], in0=csc3[:, :, 0:w_],
                        in1=csc3[:, :, w_:2 * w_], op=AL.max)
                    w_ //= 2
                nc.vector.tensor_copy(
                    d['maxpad'][:, 3 + q * 32:35 + q * 32].rearrange(
                        "p (f o) -> p f o", o=1),
                    csc3[:, :, 0:1])

            def emit_stats_tail(s):
                d = st[s]
                # finish pixel max: 2048 -> 64 -> 1
                w_ = 1024
                while w_ >= 64:
                    nc.vector.tensor_tensor(out=d['px'][:, 0:w_],
                                            in0=d['px'][:, 0:w_],
                                            in1=d['px'][:, w_:2 * w_], op=AL.max)
                    w_ //= 2
                mx = sm.tile([C, 1], F32, tag="mx", name=f"mx{s}")
                nc.vector.reduce_max(mx[:], d['px'][:, 0:64], axis=AX.X)
                avg = sm.tile([C, 1], F32, tag="avg", name=f"avg{s}")
                nc.vector.reduce_sum(avg[:], d['avp'][:], axis=AX.X)
                # ChannelAttention MLP
                psa = psA.tile([C, CH], F32, tag="big", name=f"psa{s}")
                nc.tensor.matmul(psa[0:8, 0:1], c1a[:], avg[:], start=True, stop=True)
                ha = sm.tile([8, 1], F32, tag="ha", name=f"ha{s}")
                nc.scalar.activation(ha[:], psa[0:8, 0:1], AF.Relu)
                psm = psA.tile([C, CH], F32, tag="big", name=f"psm{s}")
                nc.tensor.matmul(psm[0:8, 0:1], c1m[:], mx[:], start=True, stop=True)
                hm = sm.tile([8, 1], F32, tag="hm", name=f"hm{s}")
                nc.scalar.activation(hm[:], psm[0:8, 0:1], AF.Relu)
                psr = psA.tile([C, CH], F32, tag="big", name=f"psr{s}")
                nc.tensor.matmul(psr[0:1, 0:C], ha[:], c2r[:], start=True, stop=False)
                nc.tensor.matmul(psr[0:1, 0:C], hm[:], c2r[:], start=False, stop=True)
                ca = sm.tile([1, C], BF16, tag="ca", name=f"ca{s}")
                nc.scalar.activation(ca[:], psr[0:1, 0:C], AF.Sigmoid)
                d['ca'] = ca
                # SpatialAttention 7x7 as banded matmuls
                pss = psA.tile([C, CH], F32, tag="big", name=f"pss{s}")
                first = True
                for chn, pad in ((0, d['sumpad']), (1, d['maxpad'])):
                    for dy in range(7):
                        nc.tensor.matmul(pss[0:128, 0:128], pad[:, dy:dy + 128],
                                         bmat[:, chn * 7 + dy, :],
                                         start=first,
                                         stop=(chn == 1 and dy == 6))
                        first = False
                sa_hw = sm.tile([128, 128], BF16, tag="sa_hw", name=f"sa_hw{s}")
                nc.scalar.activation(sa_hw[:], pss[0:128, 0:128], AF.Sigmoid)
                d['sa_hw'] = sa_hw
                d['sa_fl'] = [None] * 4

            def emit_sa_flatten(s, e):
                d = st[s]
                sfq = sflp.tile([1, QP], BF16, tag="sa_fl",
                                name=f"sa_fl{s}_{e}")
                nc.sync.dma_start(sfq[:], d['sa_hw'][32 * e:32 * (e + 1), :])
                d['sa_fl'][e] = sfq

            def emit_blend_chunk(s, k):
                d = st[s]
                slf = slice(k * CH, (k + 1) * CH)
                sfq = d['sa_fl'][k // 4]
                so = (k % 4) * CH
                # tt = sigmoid(ca (x) sa) rank-1
                ptt = psT.tile([C, CH], F32, tag="tt", name=f"ptt{s}_{k}")
                nc.tensor.matmul(ptt[:, 0:512], d['ca'][:],
                                 sfq[:, so:so + 512],
                                 start=True, stop=True)
                nc.tensor.matmul(ptt[:, 512:1024], d['ca'][:],
                                 sfq[:, so + 512:so + 1024],
                                 start=True, stop=True)
                tt = ttp.tile([C, CH], BF16, tag="tt_sb", name=f"tt{s}_{k}")
                nc.scalar.activation(tt[:], ptt[:], AF.Sigmoid)
                # dt = (fv - fi) * tt
                dtmp = dtp.tile([C, CH], BF16, tag="dtmp", name=f"dtmp{s}_{k}")
                nc.vector.tensor_tensor(out=dtmp[:], in0=FV(s, slf),
                                        in1=FI(s, slf), op=AL.subtract)
                pool_mul = False
                if pool_mul:
                    nc.gpsimd.tensor_tensor(out=dtmp[:], in0=dtmp[:], in1=tt[:],
                                            op=AL.mult)
                else:
                    nc.vector.tensor_tensor(out=dtmp[:], in0=dtmp[:], in1=tt[:],
                                            op=AL.mult)
                # out = dga@fv + I@fi + dgo@dt
                pb = psA.tile([C, CH], F32, tag="big", name=f"pb{s}_{k}")
                for hf in range(2):
                    fs = slice(k * CH + hf * 512, k * CH + (hf + 1) * 512)
                    po = pb[:, hf * 512:(hf + 1) * 512]
                    nc.tensor.matmul(po, d['dga'][:], FV(s, fs), start=True, stop=False)
                for hf in range(2):
                    fs = slice(k * CH + hf * 512, k * CH + (hf + 1) * 512)
                    po = pb[:, hf * 512:(hf + 1) * 512]
                    nc.tensor.matmul(po, eye[:], FI(s, fs), start=False, stop=False)
                for hf in range(2):
                    ms = slice(hf * 512, (hf + 1) * 512)
                    po = pb[:, hf * 512:(hf + 1) * 512]
                    nc.tensor.matmul(po, d['dgo'][:], dtmp[:, ms], start=False, stop=True)
                ob = obp.tile([C, CH], BF16, tag="ob", name=f"ob{s}_{k}")
                act_evac = not (s == 1 and k % 4 == 2)
                if act_evac:
                    nc.scalar.copy(ob[:], pb[:])
                else:
                    nc.vector.tensor_scalar(ob[:], pb[:], 1.0, 0.0, AL.mult, AL.add)
                nc.sync.dma_start(d_out[s][:, slf], ob[:])

            # ================= schedule =================
            emit_alloc(0)
            for q in range(4):
                emit_load_q(0, q)
                emit_pool_q(0, q)
            emit_amlp(0)
            emit_alloc(1)
            emit_load_q(1, 0)
            emit_load_q(1, 1)
            for q in range(4):
                for c in range(4 * q, 4 * q + 4):
                    emit_conv_chunk(0, c)
                emit_mean_q(0, q)
                emit_stats_q(0, q)
            emit_pool_q(1, 0)
            emit_pool_q(1, 1)
            emit_load_q(1, 2)
            emit_pool_q(1, 2)
            emit_load_q(1, 3)
            emit_pool_q(1, 3)
            emit_mean_evac(0)
            emit_stats_tail(0)
            emit_sa_flatten(0, 0)
            emit_sa_flatten(0, 1)
            for k in range(0, 4):
                emit_blend_chunk(0, k)
            emit_sa_flatten(0, 2)
            for k in range(4, 8):
                emit_blend_chunk(0, k)
            emit_sa_flatten(0, 3)
            emit_blend_chunk(0, 8)
            emit_amlp(1)
            emit_blend_chunk(0, 9)
            emit_blend_chunk(0, 10)
            cq = 0
            for k in range(11, 16):
                if cq < 4:
                    for c in range(4 * cq, 4 * cq + 4):
                        emit_conv_chunk(1, c)
                    emit_stats_q(1, cq)
                    cq += 1
                emit_blend_chunk(0, k)
            while cq < 4:
                for c in range(4 * cq, 4 * cq + 4):
                    emit_conv_chunk(1, c)
                emit_stats_q(1, cq)
                cq += 1
            for q in range(4):
                emit_mean_q(1, q)
            emit_mean_evac(1)
            emit_stats_tail(1)
            emit_sa_flatten(1, 0)
            emit_sa_flatten(1, 1)
            for k in range(16):
                emit_blend_chunk(1, k)
                if k == 3:
                    emit_sa_flatten(1, 2)
                if k == 7:
                    emit_sa_flatten(1, 3)

    nc.compile()
    return nc


def _host_consts(ca1_w, ca1_b, bn_a_g, bn_a_b, bn_a_m, bn_a_v,
                 ca2_w, ca2_b, bn_b_g, bn_b_b, bn_b_m, bn_b_v,
                 conv1_w, conv1_b, bn_c_g, bn_c_b, bn_c_m, bn_c_v,
                 chatt_w1, chatt_w2, sa_w):
    bf = ml_dtypes.bfloat16
    f = np.float32
    k_a = bn_a_g / np.sqrt(bn_a_v + EPS)
    w1 = ca1_w * k_a[:, None]
    b1 = (ca1_b - bn_a_m) * k_a + bn_a_b
    k_b = bn_b_g / np.sqrt(bn_b_v + EPS)
    w2 = ca2_w * k_b[:, None]
    b2 = (ca2_b - bn_b_m) * k_b + bn_b_b
    s_c = bn_c_g / np.sqrt(bn_c_v + EPS)
    b_c = (conv1_b - bn_c_m) * s_c + bn_c_b
    # conv1 weights with BN scale folded (scales output channel o)
    wv = conv1_w[:, :C] * s_c[:, None]
    wi = conv1_w[:, C:] * s_c[:, None]
    l1v = (w1[:, :C] / P).T
    l1i = (w1[:, C:] / P).T
    bmat = np.zeros((14, 128, 128), np.float32)
    for chn in range(2):
        scale = (1.0 / 128.0) if chn == 0 else 1.0
        for dy in range(7):
            for dx in range(7):
                off = dx - 3
                v = sa_w[0, chn, dy, dx] * scale
                if off >= 0:
                    idx = np.arange(0, 128 - off)
                    bmat[chn * 7 + dy, idx + off, idx] = v
                else:
                    idx = np.arange(-off, 128)
                    bmat[chn * 7 + dy, idx + off, idx] = v
    return {
        "wvi_t": np.ascontiguousarray(wv.T).astype(bf),
        "wir_t": np.ascontiguousarray(wi.T).astype(bf),
        "eye": np.eye(C, dtype=f).astype(bf),
        "bmat": bmat.astype(bf),
        "l1v": np.ascontiguousarray(l1v).astype(f),
        "l1i": np.ascontiguousarray(l1i).astype(f),
        "b1": b1.reshape(-1, 1).astype(f),
        "l2": np.ascontiguousarray(w2.T).astype(f),
        "b2": b2.reshape(-1, 1).astype(f),
        "bc": b_c.reshape(-1, 1).astype(f),
        "c1a": np.ascontiguousarray((chatt_w1 / P).T).astype(f),
        "c1m": np.ascontiguousarray(chatt_w1.T).astype(f),
        "c2r": np.ascontiguousarray(chatt_w2.T).astype(f),
        "one_col": np.ones((C, 1), f).astype(bf),
    }


def kernel(f_vi, f_ir, ca1_w, ca1_b, bn_a_g, bn_a_b, bn_a_m, bn_a_v,
           ca2_w, ca2_b, bn_b_g, bn_b_b, bn_b_m, bn_b_v,
           conv1_w, conv1_b, bn_c_g, bn_c_b, bn_c_m, bn_c_v,
           chatt_w1, chatt_w2, sa_w, _trace=False):
    if "nc" not in _cache:
        _cache["nc"] = _build_program()
    nc = _cache["nc"]

    consts = _host_consts(
        np.asarray(ca1_w, np.float32), np.asarray(ca1_b, np.float32),
        np.asarray(bn_a_g, np.float32), np.asarray(bn_a_b, np.float32),
        np.asarray(bn_a_m, np.float32), np.asarray(bn_a_v, np.float32),
        np.asarray(ca2_w, np.float32), np.asarray(ca2_b, np.float32),
        np.asarray(bn_b_g, np.float32), np.asarray(bn_b_b, np.float32),
        np.asarray(bn_b_m, np.float32), np.asarray(bn_b_v, np.float32),
        np.asarray(conv1_w, np.float32), np.asarray(conv1_b, np.float32),
        np.asarray(bn_c_g, np.float32), np.asarray(bn_c_b, np.float32),
        np.asarray(bn_c_m, np.float32), np.asarray(bn_c_v, np.float32),
        np.asarray(chatt_w1, np.float32), np.asarray(chatt_w2, np.float32),
        np.asarray(sa_w, np.float32))

    fv = np.asarray(f_vi, np.float32).reshape(N, C, P)
    fi = np.asarray(f_ir, np.float32).reshape(N, C, P)
    in_maps = []
    for i in range(NCORES):
        m = dict(consts)
        m["f_vi"] = np.ascontiguousarray(fv[i * SPC:(i + 1) * SPC])
        m["f_ir"] = np.ascontiguousarray(fi[i * SPC:(i + 1) * SPC])
        in_maps.append(m)

    res = bass_utils.run_bass_kernel_spmd(nc, in_maps, core_ids=list(range(NCORES)),
                                          trace=_trace)
    if _trace:
        _cache["last_trace"] = res
    out = np.concatenate(
        [np.asarray(res.results[i]["out"]).astype(np.float32)
         for i in range(NCORES)], axis=0)
    return out.reshape(N, C, H, W)

